# revision 33
# baseline (speedup 1.0000x reference)
"""Trainium2 Bass kernel for nn_GAT_7851200217746 (hierarchical GAT message passing).

Algorithm (aggregate-first GAT restructuring, v3):
  For each GAT layer on (x_self [G,F], x_neigh [G,E,F], W, a_s, a_n):
    w_s[f,h] = sum_d W[f,h*D+d] a_s[h,d];  w_n likewise
    e_s = x_self @ w_s;  e_n = x_neigh @ w_n
    alpha = softmax_E(leaky_relu(e_s + e_n))
    x_agg[g,h,:] = sum_e alpha[g,e,h] x_neigh[g,e,:]    (aggregate in INPUT space)
    out[g, h*D:(h+1)*D] = x_agg[g,h,:] @ W[:, h*D:(h+1)*D]

v3 structural changes vs v2:
  - GAT layers are linear after the attention weighting, so layer-1
    attention combos are fused through W0: Cn[f,h1,h] = sum_d W0[f,h1D+d]
    w1n[h1D+d,h].  Stage-C logits come straight from x_aggA^T (xa_bf)
    with Cn as the moving operand; e_sC likewise from stage-B's xb_bf
    with Cs.  h1_new^T (h1t), h0_new (h0t) and all PE transposes are
    GONE, as is the 5.5us gpsimd memset of h1t.
  - h1_new row-major is produced directly: stationary = xa_bf 80-group
    chunk (out partitions = group rows), moving = W0 head slice.  The
    same stationary feeds the e_nC matmul (two MMs per weight load).
  - Skew-2 software pipeline: per iteration the PE issues
    L(s) logits | gsum(s-1) | Cagg(s-3) | H-front(s-2) | G(s-1) agg |
    gsumC(s-2), so ~4us of independent PE work covers every DVE/ACT
    softmax chain: no PE idle gaps, HAM stays at K=8/8.
  - Elementwise work spread across DVE / ACT / GpSimd by throughput.
  - Streams are per-superiter DMAs (16 x ~1MB/0.5MB) issued upfront in
    consumption order on two queues; smalls on the gpsimd queue.
Sharding: pure data-parallel over batch (128 batches/core x 8 cores).
"""

import sys

sys.path.insert(0, "/opt/trn_rl_repo")

from contextlib import ExitStack

import ml_dtypes
import numpy as np

import concourse.bass as bass
import concourse.tile as tile
from concourse import bacc, mybir
import concourse.bass_utils as bass_utils

BF = mybir.dt.bfloat16
F32 = mybir.dt.float32
FP8 = mybir.dt.float8e4
AF = mybir.ActivationFunctionType
ALU = mybir.AluOpType

NCORES = 8
B, FEAT, HID, OUT, H = 1024, 128, 512, 256, 4
BC = B // NCORES              # 128 batches per core
G1 = BC * 10                  # 1280 level-1 groups (h1 rows)
R2 = G1 * 25                  # 32000 h2 rows
TR = 125                      # stage-A tile rows (5 groups of 25)
TPS = 32                      # tiles per superiter
NSUP = R2 // (TR * TPS)       # 8 superiters
SUPR = TR * TPS               # 4000 rows per superiter
SUPG = SUPR // 25             # 160 groups per superiter
X2TP = 4032                   # x2t cols per superiter incl zero pad
G1P = 1344                    # x1t padded cols (>= 1200+128)
TB = 80                       # stage-B/C tile rows (8 groups of 10)
NTB = G1 // TB                # 16 tiles
LEAKY = 0.2

# packed bf16 const/per-core "smalls" layout: name -> n_cols.  Split into
# three DMA chunks: c1 = weights/indicators + x1tp + x0t (pre-loop PE
# work), c2 = x1r (stage-B agg, iter 0), c3 = big layer weights (needed
# from H(0) / epilogue).
SMALLS = [
    # --- chunk 1 ---
    ("w0s4", H), ("w0n4", H),
    ("cn16", 4 * H), ("cs16", 4 * H),     # [128, 4, H] fused L1 combos
    ("e5p", 5),               # row->group indicator, rows>=125 zero
    ("e5xp", 128),            # group-sum expander + identity pad
    ("L80p", 128),            # esam expander [g' mod 5 == r div 25]
    ("LBp", 128),             # stage-B/C expander [b mod 8 == r div 10]
    ("ind16", 16),            # [g' div 5 == t'] (rows>=80 zero)
    ("indB16", 16),           # [b div 8 == i]
    ("i128b", 128),           # identity (output transpose)
    ("e10p", 8),              # stage-B/C row->group indicator (rows>=80 zero)
    ("e10xp", 128),           # stage-B/C group-sum expander + identity pad
    ("x1tp", G1P),            # h1^T feature-major, padded with zeros
    ("x0t", BC),              # h0^T feature-major
    # --- chunk 2 ---
    ("x1r", NTB * FEAT),      # h1 row-major tiles [80 rows used, pad 0]
    # --- chunk 3 ---
    ("w0b", HID),                          # W0 bf16
    ("w1b", 4 * HID),                      # [128, 4, 512]
    ("wfcb", 4 * OUT),                     # [128, 4, 256]
]
SOFF = {}
_off = 0
for _n, _c in SMALLS:
    SOFF[_n] = _off
    _off += _c
SCOLS = _off
SC0 = SOFF["x1tp"]            # end of weights+indicators
SC1 = SOFF["x1r"]             # end of chunk 1
SC2 = SOFF["w0b"]             # end of chunk 2
SC3 = SOFF["w1b"]             # end of w0b


def build_program(debug_out=False):
    nc = bacc.Bacc(
        "TRN2",
        target_bir_lowering=False,
        debug=False,
        enable_asserts=False,
        num_devices=NCORES,
    )

    # per-superiter stream tensors, sup-major so each DMA is a fully
    # contiguous 2-dim [128, ~4-8KB] transfer (spreads descriptors over
    # all 16 SDMA engines).
    x2r = nc.dram_tensor("x2r", [NSUP // 2 * 128, 2 * TPS * FEAT], BF,
                         kind="ExternalInput").ap()
    x2t8 = nc.dram_tensor("x2t8", [NSUP // 2 * FEAT, 2 * X2TP], FP8,
                          kind="ExternalInput").ap()
    smalls = nc.dram_tensor("smalls", [128, SCOLS], BF,
                            kind="ExternalInput").ap()
    out_d = nc.dram_tensor("out", [BC, OUT], F32, kind="ExternalOutput").ap()

    with tile.TileContext(nc) as tc, ExitStack() as ctx:
        const = ctx.enter_context(tc.tile_pool(name="const", bufs=1))
        perst = ctx.enter_context(tc.tile_pool(name="perst", bufs=1))
        stream = ctx.enter_context(tc.tile_pool(name="stream", bufs=1))
        sm = ctx.enter_context(tc.tile_pool(name="sm", bufs=2))
        h1p = ctx.enter_context(tc.tile_pool(name="h1p", bufs=4))
        # PSUM: 8 bank-slots of 2KB.  en x2 + agg x2 + h1rps x2 + cen x2.
        psA = ctx.enter_context(tc.tile_pool(name="psA", bufs=2, space="PSUM"))
        psb = ctx.enter_context(tc.tile_pool(name="psb", bufs=2, space="PSUM"))
        psC = ctx.enter_context(tc.tile_pool(name="psC", bufs=2, space="PSUM"))

        sm_s = const.tile([128, SCOLS], BF, name="smalls")

        def sv(name, split=None):
            c = dict(SMALLS)[name]
            v = sm_s[:, SOFF[name]:SOFF[name] + c]
            if split is not None:
                v = v.rearrange("p (a b) -> p a b", a=split)
            return v

        x1tp = sv("x1tp")
        x0t = sv("x0t")
        x1r = sv("x1r", NTB)
        w0s4 = sv("w0s4")
        w0n4 = sv("w0n4")
        cn16 = sv("cn16", 4)
        cs16 = sv("cs16", 4)
        w0b = sv("w0b")
        w1b = sv("w1b", 4)
        wfcb = sv("wfcb", 4)
        e5p = sv("e5p")
        e5xp = sv("e5xp")
        L80p = sv("L80p")
        LBp = sv("LBp")
        ind16 = sv("ind16")
        indB16 = sv("indB16")
        i128b = sv("i128b")
        e10p = sv("e10p")
        e10xp = sv("e10xp")

        xc_sb = perst.tile([128, 4, NTB, 8, H], BF)  # stage-C agg (d-major)
        xb_bf = perst.tile([128, NTB, 8, H], BF)    # stage-B agg, f-major
        R_all = perst.tile([128, 16, 16, H], BF)    # stage-A e_s expander rhs
        RB = perst.tile([128, 16, H], BF)           # stage-B e_s expander rhs
        RC = perst.tile([128, 16, H], BF)           # stage-C e_s expander rhs

        x2t_t = [stream.tile([FEAT, 2, X2TP], FP8, tag=f"x2t{q}",
                             name=f"x2t{q}") for q in range(NSUP // 2)]
        x2r_t = [stream.tile([128, 2, TPS, FEAT], BF, tag=f"x2r{q}",
                             name=f"x2r{q}") for q in range(NSUP // 2)]

        mm = nc.tensor.matmul
        x2r_v = x2r.rearrange("(q p) c -> q p c", q=NSUP // 2)
        x2t_v = x2t8.rearrange("(q p) c -> q p c", q=NSUP // 2)

        # ---------- all DMAs upfront, consumption order ----------
        # 4 queues in parallel: SDMA engines round-robin between queues at
        # packet granularity, so concurrent queues hide per-descriptor HBM
        # latency.  smalls lead on the HWDGE queues (sync/scalar) so the
        # pre-loop PE work starts ASAP; x2r goes as 2MB pair-DMAs (16KB
        # descriptors).
        def dma_t(eng, s):
            eng.dma_start(x2t_t[s][:], x2t_v[s])

        def dma_r(eng, s):
            # one superiter = half of a pair tile; 8KB descriptors
            q, s2 = s // 2, s % 2
            eng.dma_start(
                x2r_t[q][:, s2, :, :].rearrange("p a b -> p (a b)"),
                x2r_v[q][:, s2 * TPS * FEAT:(s2 + 1) * TPS * FEAT])

        # Per-superiter DMAs sliced out of the pair tiles, interleaved
        # across the two HWDGE queues in consumption order (empirically the
        # fastest layout measured); non-critical smalls trail on gpsimd.
        def dma_t(eng, s):
            q, s2 = s // 2, s % 2
            eng.dma_start(x2t_t[q][:, s2, :],
                          x2t_v[q][:, s2 * X2TP:(s2 + 1) * X2TP])

        def dma_r(eng, s):
            q, s2 = s // 2, s % 2
            eng.dma_start(
                x2r_t[q][:, s2, :, :].rearrange("p a b -> p (a b)"),
                x2r_v[q][:, s2 * TPS * FEAT:(s2 + 1) * TPS * FEAT])

        def dma_tp(eng, q):
            eng.dma_start(x2t_t[q][:].rearrange("p a b -> p (a b)"),
                          x2t_v[q])

        # t0/t1 lead the scalar queue so L(0)/L(1) never wait behind the
        # smalls transfer on sync; r0 takes sync position 2.  Both queues
        # stay monotone in consumption order.
        nc.sync.dma_start(sm_s[:, :SC1], smalls[:, :SC1])
        dma_t(nc.sync, 0)
        dma_r(nc.scalar, 0)
        dma_t(nc.sync, 1)
        dma_r(nc.sync, 1)
        dma_t(nc.scalar, 2)
        dma_r(nc.scalar, 2)
        dma_t(nc.sync, 3)
        dma_r(nc.sync, 3)
        dma_t(nc.scalar, 4)
        dma_r(nc.scalar, 4)
        dma_t(nc.sync, 5)
        dma_r(nc.sync, 5)
        dma_t(nc.scalar, 6)
        dma_r(nc.scalar, 6)
        dma_t(nc.sync, 7)
        dma_r(nc.sync, 7)
        nc.gpsimd.dma_start(sm_s[:, SC1:SC2], smalls[:, SC1:SC2])
        nc.gpsimd.dma_start(sm_s[:, SC2:], smalls[:, SC2:])

        # ---------- pre-loop: stage-A e_s, stage-B e_s/e_n ----------
        # es80 chunk c: groups [80c, 80c+80).  R_all[g',c,t',h] =
        # e_s[80c+g', h] * [g' div 5 == t']
        es80_sb = sm.tile([128, 16, H], BF, tag="es80", bufs=1)
        for half in range(2):
            es_ps = psC.tile([128, 8, H], F32, tag="cen", name=f"es_ps{half}")
            for c_ in range(8):
                c = 8 * half + c_
                mm(es_ps[:, c_, :], x1tp[:, 80 * c:80 * c + 128], w0s4,
                   start=True, stop=True, skip_group_check=True)
            nc.vector.tensor_copy(es80_sb[:, 8 * half:8 * (half + 1), :],
                                  es_ps[:])
        nc.vector.tensor_mul(
            R_all[:],
            es80_sb[:].unsqueeze(2).broadcast_to((128, 16, 16, H)),
            ind16.unsqueeze(1).unsqueeze(3).broadcast_to((128, 16, 16, H)),
        )

        # esB = h0 @ w0s (one mm), RB = esB * indB16
        esB_ps = psC.tile([128, H], F32, tag="cen", name="esB_ps")
        mm(esB_ps[:], x0t, w0s4, start=True, stop=True, skip_group_check=True)
        esB_sb = sm.tile([128, H], BF, tag="esB", bufs=1)
        nc.vector.tensor_copy(esB_sb[:], esB_ps[:])
        nc.vector.tensor_mul(
            RB[:],
            esB_sb[:].unsqueeze(1).broadcast_to((128, 16, H)),
            indB16.unsqueeze(2).broadcast_to((128, 16, H)),
        )

        # stage-B logits
        enb_t = psA.tile([128, NTB, H], F32, tag="en", name="enb")
        mm(enb_t[:], LBp, RB[:], start=True, stop=False,
           skip_group_check=True)
        for i in range(NTB):
            mm(enb_t[:, i, :], x1tp[:, TB * i:TB * i + 128], w0n4,
               start=False, stop=True, skip_group_check=True)
        lrB = sm.tile([128, NTB, H], F32, tag="lr")
        nc.vector.tensor_scalar_mul(lrB[:], enb_t[:], LEAKY)
        nc.vector.tensor_max(lrB[:], lrB[:], enb_t[:])
        pB = sm.tile([128, NTB, H], BF, tag="p")
        nc.scalar.activation(pB[:], lrB[:], AF.Exp)
        mm(enb_t[:], e10xp, pB[:], start=True, stop=True,
           skip_group_check=True)
        rcB = sm.tile([128, NTB, H], F32, tag="rc")
        nc.vector.reciprocal_approx_fast(rcB[:], enb_t[:])
        alB = sm.tile([128, NTB, H], BF, tag="al")
        nc.gpsimd.tensor_mul(alB[:], pB[:], rcB[:])
        albdB = sm.tile([128, NTB, 8, H], BF, tag="albd")
        nc.gpsimd.tensor_mul(
            albdB[:],
            alB[:].unsqueeze(2).broadcast_to((128, NTB, 8, H)),
            e10p.unsqueeze(1).unsqueeze(3).broadcast_to((128, NTB, 8, H)),
        )

        # ---------- stage-A per-phase helpers ----------
        en_ts = [None] * NSUP     # logits psum tiles
        p_ts = [None] * NSUP      # exp'd logits
        albd_ts = [None] * NSUP
        xa_ts = [None] * NSUP     # x_aggA^T bf16
        h1r_ts = [None] * NSUP    # [chunk0, chunk1] row-major h1_new
        encp_ts = [None] * NSUP
        pc_ts = [None] * NSUP
        albdc_ts = [None] * NSUP

        def phase_L(s):
            en_t = psA.tile([128, TPS, H], F32, tag="en", name=f"en{s}")
            mm(en_t[:], L80p, R_all[:, 2 * s:2 * s + 2, :, :],
               start=True, stop=False, skip_group_check=True)
            for t in range(TPS):
                mm(en_t[:, t, :], x2t_t[s // 2][:, s % 2, 125 * t:125 * t + 128],
                   w0n4, start=False, stop=True, skip_group_check=True)
            en_ts[s] = en_t
            # softmax front half: leaky_relu (DVE) + exp (ACT), t-halved
            # so each half chains to the group-sum independently
            lr = sm.tile([128, TPS, H], F32, tag="lr", name=f"lr{s}")
            p = sm.tile([128, TPS, H], BF, tag="p", name=f"p{s}")
            for hl in range(2):
                t0 = slice(16 * hl, 16 * (hl + 1))
                nc.vector.tensor_scalar_mul(lr[:, t0], en_t[:, t0], LEAKY)
                nc.vector.tensor_max(lr[:, t0], lr[:, t0], en_t[:, t0])
                nc.scalar.activation(p[:, t0], lr[:, t0], AF.Exp)
            p_ts[s] = p

        def phase_gsum(s):
            # group-sum reuses the logits PSUM region (logits dead after
            # exp).  Everything is t-halved: each half runs
            # gsum-mm -> recip -> al -> albd independently, so G's first
            # 16 matmuls start a full half-chain earlier (subtile deps).
            en_t = en_ts[s]
            rc = sm.tile([128, TPS, H], F32, tag="rc", name=f"rc{s}")
            al = sm.tile([128, TPS, H], BF, tag="al", name=f"al{s}")
            albd = sm.tile([128, TPS, 5, H], BF, tag="albd", name=f"albd{s}")
            HT = TPS // 2
            for hl in range(2):
                tsl = slice(HT * hl, HT * (hl + 1))
                mm(en_t[:, tsl], e5xp, p_ts[s][:, tsl], start=True,
                   stop=True, skip_group_check=True)
                nc.vector.reciprocal_approx_fast(rc[:, tsl], en_t[:, tsl])
                nc.gpsimd.tensor_mul(al[:, tsl], p_ts[s][:, tsl], rc[:, tsl])
                nc.gpsimd.tensor_mul(
                    albd[:, tsl],
                    al[:, tsl].unsqueeze(2).broadcast_to((128, HT, 5, H)),
                    e5p.unsqueeze(1).unsqueeze(3).broadcast_to(
                        (128, HT, 5, H)),
                )
            albd_ts[s] = albd

        def phase_G(s):
            # aggregation: x_agg^T[f, (t, g, h)]
            albd = albd_ts[s]
            xa_bf = sm.tile([128, TPS, 5, H], BF, tag="xabf", name=f"xa{s}")
            for j in range(2):
                xa_ps = psb.tile([128, TPS // 2, 20], F32, tag="agg",
                                 name=f"xaps{s}_{j}")
                for t2 in range(16):
                    t = 16 * j + t2
                    mm(xa_ps[:, t2, :], x2r_t[s // 2][:TR, s % 2, t, :],
                       albd[:TR, t, :, :], start=True, stop=True,
                       skip_group_check=True)
                nc.scalar.copy(
                    xa_bf[:, 16 * j:16 * (j + 1), :, :].rearrange(
                        "p a b c -> p (a b c)"),
                    xa_ps[:].rearrange("p t x -> p (t x)"))
            xa_ts[s] = xa_bf

        def phase_Hfront(s):
            # h1_new row-major directly from xa_bf chunks (stationary),
            # with the stage-C e_n matmuls sharing each weight load.
            xa_bf = xa_ts[s]
            encp = psC.tile([128, 2, H], F32, tag="cen", name=f"encp{s}")
            mm(encp[:], LBp, RC[:, 2 * s:2 * s + 2, :], start=True,
               stop=False, skip_group_check=True)
            h1r_ts[s] = []
            for c in range(2):
                h1r_ps = psb.tile([128, H, 128], F32, tag="h1rps",
                                  name=f"h1rps{s}_{c}")
                for h1 in range(H):
                    stat = xa_bf[:, 16 * c:16 * (c + 1), :, h1]
                    mm(h1r_ps[:TB, h1, :], stat,
                       w0b[:, 128 * h1:128 * (h1 + 1)],
                       start=True, stop=True, skip_group_check=True)
                    mm(encp[:TB, c, :], stat, cn16[:, h1, :],
                       start=False, stop=(h1 == H - 1),
                       skip_group_check=True)
                h1r_sb = h1p.tile([128, H, 128], BF, tag="h1rsb",
                                  name=f"h1rsb{s}_{c}")
                nc.vector.tensor_copy(h1r_sb[:TB], h1r_ps[:TB])
                h1r_ts[s].append(h1r_sb)
            encp_ts[s] = encp
            lrc = sm.tile([128, 2, H], F32, tag="lrc", name=f"lrc{s}")
            nc.vector.tensor_scalar_mul(lrc[:], encp[:], LEAKY)
            nc.vector.tensor_max(lrc[:], lrc[:], encp[:])
            pc = sm.tile([128, 2, H], BF, tag="pc", name=f"pc{s}")
            nc.scalar.activation(pc[:], lrc[:], AF.Exp)
            pc_ts[s] = pc

        def phase_gsumC(s):
            encp = encp_ts[s]
            mm(encp[:], e10xp, pc_ts[s][:], start=True, stop=True,
               skip_group_check=True)
            rcc = sm.tile([128, 2, H], F32, tag="rcc", name=f"rcc{s}")
            nc.vector.reciprocal_approx_fast(rcc[:], encp[:])
            alc = sm.tile([128, 2, H], BF, tag="alc", name=f"alc{s}")
            nc.gpsimd.tensor_mul(alc[:], pc_ts[s][:], rcc[:])
            albdc = sm.tile([128, 2, 8, H], BF, tag="albdc",
                            name=f"albdc{s}")
            nc.gpsimd.tensor_mul(
                albdc[:],
                alc[:].unsqueeze(2).broadcast_to((128, 2, 8, H)),
                e10p.unsqueeze(1).unsqueeze(3).broadcast_to((128, 2, 8, H)),
            )
            albdc_ts[s] = albdc

        def phase_Cagg(s):
            albdc = albdc_ts[s]
            xc_ps = psb.tile([128, 2, 4, 8, H], F32, tag="agg",
                             name=f"xcps{s}")
            for c in range(2):
                for k in range(4):
                    mm(xc_ps[:, c, k, :, :], h1r_ts[s][c][:TB, k, :],
                       albdc[:TB, c, :, :], start=True, stop=True,
                       skip_group_check=True)
            nc.vector.tensor_copy(
                xc_sb[:, :, 2 * s:2 * s + 2, :, :].transpose([0, 2, 1, 3, 4]),
                xc_ps[:])

        # ---------- the skew-2 pipeline ----------
        for s in range(NSUP):
            phase_L(s)
            if s == 1:
                # stage-C e_s from xb_bf via fused Cs combos
                esC_ps = psC.tile([128, H], F32, tag="cen", name="esC_ps")
                for h1 in range(H):
                    mm(esC_ps[:], xb_bf[:, :, :, h1], cs16[:, h1, :],
                       start=(h1 == 0), stop=(h1 == H - 1),
                       skip_group_check=True)
                esC_sb = sm.tile([128, H], BF, tag="esC", bufs=1)
                nc.vector.tensor_copy(esC_sb[:], esC_ps[:])
                nc.vector.tensor_mul(
                    RC[:],
                    esC_sb[:].unsqueeze(1).broadcast_to((128, 16, H)),
                    indB16.unsqueeze(2).broadcast_to((128, 16, H)),
                )
            if s >= 1:
                phase_gsum(s - 1)
            if s >= 3:
                phase_Cagg(s - 3)
            if s >= 2:
                phase_Hfront(s - 2)
            if s == 0:
                # stage-B aggregation (albdB ready during L(0))
                xb_ps = psb.tile([128, NTB, 8, H], F32, tag="h1rps",
                                 name="xb_ps")
                for i in range(NTB):
                    mm(xb_ps[:, i, :, :], x1r[:, i, :], albdB[:, i, :, :],
                       start=True, stop=True, skip_group_check=True)
                nc.scalar.copy(xb_bf[:], xb_ps[:])
            if s >= 1:
                phase_G(s - 1)
            if s >= 2:
                phase_gsumC(s - 2)

        # ---------- drain + split epilogue ----------
        # Only batches 112-127 (xc tiles 14,15) depend on the last
        # superiter: run the epilogue for batches 0-111 concurrently with
        # the s=7 drain phases, and a small tail for the rest.
        # epilogue PSUM lives in tags that are DEAD during the drain:
        # en (psA) frees after gsum(7); or_ps slots into the agg rotation
        # after xc(6).
        hf_bf = sm.tile([128, H, BC], BF, tag="hfbf", bufs=1)
        ot_bf = sm.tile([128, 2, BC], BF, tag="otbf", bufs=1)
        or_sb = sm.tile([BC, 2, 128], F32, tag="orsb", bufs=1)

        def epilogue(b0, b1, t0, t1):
            # hf/FC columns [b0:b1); output transpose + store rows [t0:t1)
            # (transpose out base partition must be 0/32/64, so the
            # early/late split differs between the two halves).
            for h in range(H):
                for k in range(4):
                    mm(hf_ps[:, h, b0:b1], w1b[:, k, 128 * h:128 * (h + 1)],
                       xc_sb[:, k, :, :, h].rearrange(
                           "p a b -> p (a b)")[:, b0:b1],
                       start=(k == 0), stop=(k == 3), skip_group_check=True)
            nc.scalar.copy(hf_bf[:, :, b0:b1], hf_ps[:, :, b0:b1])
            for m in range(2):
                for k in range(4):
                    mm(of_ps[:, m, b0:b1], wfcb[:, k, 128 * m:128 * (m + 1)],
                       hf_bf[:, k, b0:b1], start=(k == 0), stop=(k == 3),
                       skip_group_check=True)
            nc.vector.tensor_copy(ot_bf[:, :, b0:b1], of_ps[:, :, b0:b1])
            for m in range(2):
                mm(or_ps[t0:t1, m, :], ot_bf[:, m, t0:t1], i128b,
                   start=True, stop=True, skip_group_check=True)
            nc.vector.tensor_copy(or_sb[t0:t1], or_ps[t0:t1])
            nc.sync.dma_start(
                out_d.rearrange("b (m o) -> b m o", m=2)[t0:t1],
                or_sb[t0:t1])

        phase_gsum(NSUP - 1)
        phase_Cagg(NSUP - 3)
        phase_Hfront(NSUP - 2)
        phase_G(NSUP - 1)
        phase_gsumC(NSUP - 2)
        phase_Cagg(NSUP - 2)
        hf_ps = psA.tile([128, H, BC], F32, tag="en", name="hf_ps")
        of_ps = psA.tile([128, 2, BC], F32, tag="en", name="of_ps")
        or_ps = psb.tile([BC, 2, 128], F32, tag="agg", name="or_ps")
        epilogue(0, 112, 0, 64)
        phase_Hfront(NSUP - 1)
        phase_gsumC(NSUP - 1)
        phase_Cagg(NSUP - 1)
        epilogue(112, 128, 64, 128)

        if debug_out:
            dbg_xb = nc.dram_tensor("dbg_xb", [128, NTB * 8 * H], BF,
                                    kind="ExternalOutput").ap()
            dbg_xc = nc.dram_tensor("dbg_xc", [128, 4 * NTB * 8 * H], BF,
                                    kind="ExternalOutput").ap()
            dbg_h1r = nc.dram_tensor("dbg_h1r", [128, H * 128], BF,
                                     kind="ExternalOutput").ap()
            nc.sync.dma_start(
                dbg_xb.rearrange("p (a b c) -> p a b c", a=NTB, b=8), xb_bf[:])
            nc.sync.dma_start(
                dbg_xc.rearrange("p (a b c d) -> p a b c d", a=4, b=NTB, c=8),
                xc_sb[:])
            nc.sync.dma_start(
                dbg_h1r.rearrange("p (a b) -> p a b", a=H), h1r_ts[0][0][:])

    nc.compile()
    return nc


def _host_prep(h0, h1, h2, W0, a0_s, a0_n, W1, a1_s, a1_n, W_fc):
    bf16 = ml_dtypes.bfloat16
    fp8 = ml_dtypes.float8_e4m3
    f32 = np.float32

    def combo(W, a):  # [F, H*D], [H, D] -> [F, H]
        F_ = W.shape[0]
        return np.einsum("fhd,hd->fh", W.reshape(F_, H, 128), a).astype(f32)

    w0s = combo(W0, a0_s)
    w0n = combo(W0, a0_n)
    w1s = combo(W1, a1_s)                                         # [512, H]
    w1n = combo(W1, a1_n)
    # fused layer-1 combos: Cn[f, h1, h] = sum_d W0[f,h1*128+d] w1n[h1*128+d,h]
    W0r = np.asarray(W0, f32).reshape(FEAT, H, 128)
    cn = np.einsum("fad,adh->fah", W0r, w1n.reshape(H, 128, H))   # [128,4,H]
    cs = np.einsum("fad,adh->fah", W0r, w1s.reshape(H, 128, H))

    ar = np.arange
    sm_shared = {}

    def put(name, arr):
        a = np.zeros((128, dict(SMALLS)[name]), dtype=bf16)
        a[:arr.shape[0], :arr.shape[1]] = arr.astype(bf16)
        sm_shared[name] = a

    put("w0s4", w0s)
    put("w0n4", w0n)
    put("cn16", cn.reshape(128, 4 * H))
    put("cs16", cs.reshape(128, 4 * H))
    put("w0b", W0.astype(f32))
    put("w1b", W1.reshape(4, 128, HID).transpose(1, 0, 2).reshape(128, -1))
    put("wfcb", W_fc.reshape(4, 128, OUT).transpose(1, 0, 2).reshape(128, -1))
    e5p = (ar(128)[:, None] // 25 == ar(5)[None, :]) & (ar(128)[:, None] < 125)
    put("e5p", e5p.astype(f32))
    e5x = np.zeros((128, 128), dtype=f32)
    blk = (ar(125)[:, None] // 25 == ar(125)[None, :] // 25)
    e5x[:125, :125] = blk
    e5x[125:, :] = 0.0
    for m in range(125, 128):
        e5x[m, m] = 1.0
    put("e5xp", e5x)
    L80 = np.zeros((128, 128), dtype=f32)
    L80[:80, :125] = (ar(80)[:, None] % 5 == ar(125)[None, :] // 25)
    put("L80p", L80)
    LB = np.zeros((128, 128), dtype=f32)
    LB[:, :80] = (ar(128)[:, None] % 8 == ar(80)[None, :] // 10)
    put("LBp", LB)
    ind16 = np.zeros((128, 16), dtype=f32)
    ind16[:80] = (ar(80)[:, None] // 5 == ar(16)[None, :])
    put("ind16", ind16)
    put("indB16", (ar(128)[:, None] // 8 == ar(16)[None, :]).astype(f32))
    put("i128b", np.eye(128, dtype=f32))
    e10 = np.zeros((128, 8), dtype=f32)
    e10[:80] = (ar(80)[:, None] // 10 == ar(8)[None, :])
    put("e10p", e10)
    e10x = np.zeros((128, 128), dtype=f32)
    e10x[:80, :80] = (ar(80)[:, None] // 10 == ar(80)[None, :] // 10)
    for m in range(80, 128):
        e10x[m, m] = 1.0
    put("e10xp", e10x)

    in_maps = []
    for c in range(NCORES):
        sl = slice(c * BC, (c + 1) * BC)
        h2c = np.asarray(h2[sl], dtype=f32).reshape(R2, FEAT)
        h1c = np.asarray(h1[sl], dtype=f32).reshape(G1, FEAT)
        h0c = np.asarray(h0[sl], dtype=f32)
        m = dict(sm_shared)
        x1tp = np.zeros((128, G1P), dtype=bf16)
        x1tp[:, :G1] = h1c.T.astype(bf16)
        m["x1tp"] = x1tp
        m["x0t"] = np.zeros((128, BC), dtype=bf16)
        m["x0t"][:] = h0c.T.astype(bf16)
        x1r = np.zeros((128, NTB, FEAT), dtype=bf16)
        x1r[:TB] = h1c.astype(bf16).reshape(NTB, TB, FEAT).transpose(1, 0, 2)
        m["x1r"] = x1r.reshape(128, NTB * FEAT)
        # pack the smalls in layout order
        packed = np.concatenate([m.pop(n) for n, _ in SMALLS], axis=1)
        mm_ = {"smalls": np.ascontiguousarray(packed)}
        # x2r: [pair q, 128 partitions (125 used), 2 sup, TPS, FEAT]
        x2rt = h2c.astype(bf16).reshape(NSUP // 2, 2, TPS, TR, FEAT) \
            .transpose(0, 3, 1, 2, 4)                # [q, TR, 2, TPS, F]
        x2rp = np.zeros((NSUP // 2, 128, 2, TPS, FEAT), dtype=bf16)
        x2rp[:, :TR] = x2rt
        mm_["x2r"] = np.ascontiguousarray(
            x2rp.reshape(NSUP // 2 * 128, 2 * TPS * FEAT))
        # x2t: [pair q, FEAT, 2 sup, X2TP] feature-major fp8 (8KB rows)
        x2t = np.zeros((NSUP // 2, FEAT, 2, X2TP), dtype=fp8)
        x2t[:, :, :, :SUPR] = h2c.T.astype(fp8).reshape(
            FEAT, NSUP // 2, 2, SUPR).transpose(1, 0, 2, 3)
        mm_["x2t8"] = np.ascontiguousarray(
            x2t.reshape(NSUP // 2 * FEAT, 2 * X2TP))
        in_maps.append(mm_)
    return in_maps


_PROGRAM = None


def kernel(**inputs):
    global _PROGRAM
    if _PROGRAM is None:
        _PROGRAM = build_program()
    in_maps = _host_prep(**{k: np.asarray(v) for k, v in inputs.items()})
    res = bass_utils.run_bass_kernel_spmd(
        _PROGRAM, in_maps, core_ids=list(range(NCORES)))
    return np.concatenate([r["out"] for r in res.results], axis=0)


if __name__ == "__main__":
    build_program()
    print("program built + compiled OK")


# revision 34
# speedup vs baseline: 1.0548x; 1.0548x over previous
"""Trainium2 Bass kernel for nn_GAT_7851200217746 (hierarchical GAT message passing).

Algorithm (aggregate-first GAT restructuring, v3):
  For each GAT layer on (x_self [G,F], x_neigh [G,E,F], W, a_s, a_n):
    w_s[f,h] = sum_d W[f,h*D+d] a_s[h,d];  w_n likewise
    e_s = x_self @ w_s;  e_n = x_neigh @ w_n
    alpha = softmax_E(leaky_relu(e_s + e_n))
    x_agg[g,h,:] = sum_e alpha[g,e,h] x_neigh[g,e,:]    (aggregate in INPUT space)
    out[g, h*D:(h+1)*D] = x_agg[g,h,:] @ W[:, h*D:(h+1)*D]

v3 structural changes vs v2:
  - GAT layers are linear after the attention weighting, so layer-1
    attention combos are fused through W0: Cn[f,h1,h] = sum_d W0[f,h1D+d]
    w1n[h1D+d,h].  Stage-C logits come straight from x_aggA^T (xa_bf)
    with Cn as the moving operand; e_sC likewise from stage-B's xb_bf
    with Cs.  h1_new^T (h1t), h0_new (h0t) and all PE transposes are
    GONE, as is the 5.5us gpsimd memset of h1t.
  - h1_new row-major is produced directly: stationary = xa_bf 80-group
    chunk (out partitions = group rows), moving = W0 head slice.  The
    same stationary feeds the e_nC matmul (two MMs per weight load).
  - Skew-2 software pipeline: per iteration the PE issues
    L(s) logits | gsum(s-1) | Cagg(s-3) | H-front(s-2) | G(s-1) agg |
    gsumC(s-2), so ~4us of independent PE work covers every DVE/ACT
    softmax chain: no PE idle gaps, HAM stays at K=8/8.
  - Elementwise work spread across DVE / ACT / GpSimd by throughput.
  - Streams are per-superiter DMAs (16 x ~1MB/0.5MB) issued upfront in
    consumption order on two queues; smalls on the gpsimd queue.
Sharding: pure data-parallel over batch (128 batches/core x 8 cores).
"""

import sys

sys.path.insert(0, "/opt/trn_rl_repo")

from contextlib import ExitStack

import ml_dtypes
import numpy as np

import concourse.bass as bass
import concourse.tile as tile
from concourse import bacc, mybir
import concourse.bass_utils as bass_utils

BF = mybir.dt.bfloat16
F32 = mybir.dt.float32
FP8 = mybir.dt.float8e4
AF = mybir.ActivationFunctionType
ALU = mybir.AluOpType

NCORES = 8
B, FEAT, HID, OUT, H = 1024, 128, 512, 256, 4
BC = B // NCORES              # 128 batches per core
G1 = BC * 10                  # 1280 level-1 groups (h1 rows)
R2 = G1 * 25                  # 32000 h2 rows
TR = 125                      # stage-A tile rows (5 groups of 25)
TPS = 32                      # tiles per superiter
NSUP = R2 // (TR * TPS)       # 8 superiters
SUPR = TR * TPS               # 4000 rows per superiter
SUPG = SUPR // 25             # 160 groups per superiter
X2TP = 4032                   # x2t cols per superiter incl zero pad
G1P = 1344                    # x1t padded cols (>= 1200+128)
TB = 80                       # stage-B/C tile rows (8 groups of 10)
NTB = G1 // TB                # 16 tiles
LEAKY = 0.2

# packed bf16 const/per-core "smalls" layout: name -> n_cols.  Split into
# three DMA chunks: c1 = weights/indicators + x1tp + x0t (pre-loop PE
# work), c2 = x1r (stage-B agg, iter 0), c3 = big layer weights (needed
# from H(0) / epilogue).
SMALLS = [
    # --- chunk 1 ---
    ("w0s4", H), ("w0n4", H),
    ("cn16", 4 * H), ("cs16", 4 * H),     # [128, 4, H] fused L1 combos
    ("e5p", 5),               # row->group indicator, rows>=125 zero
    ("e5xp", 128),            # group-sum expander + identity pad
    ("L80p", 128),            # esam expander [g' mod 5 == r div 25]
    ("LBp", 128),             # stage-B/C expander [b mod 8 == r div 10]
    ("ind16", 16),            # [g' div 5 == t'] (rows>=80 zero)
    ("indB16", 16),           # [b div 8 == i]
    ("i128b", 128),           # identity (output transpose)
    ("e10p", 8),              # stage-B/C row->group indicator (rows>=80 zero)
    ("e10xp", 128),           # stage-B/C group-sum expander + identity pad
    ("x1tp", G1P),            # h1^T feature-major, padded with zeros
    ("x0t", BC),              # h0^T feature-major
    # --- chunk 2 ---
    ("x1r", NTB * FEAT),      # h1 row-major tiles [80 rows used, pad 0]
    # --- chunk 3 ---
    ("w0b", HID),                          # W0 bf16
    ("w1b", 4 * HID),                      # [128, 4, 512]
    ("wfcb", 4 * OUT),                     # [128, 4, 256]
]
SOFF = {}
_off = 0
for _n, _c in SMALLS:
    SOFF[_n] = _off
    _off += _c
SCOLS = _off
SC0 = SOFF["x1tp"]            # end of weights+indicators
SC1 = SOFF["x1r"]             # end of chunk 1
SC2 = SOFF["w0b"]             # end of chunk 2
SC3 = SOFF["w1b"]             # end of w0b


def build_program(debug_out=False):
    nc = bacc.Bacc(
        "TRN2",
        target_bir_lowering=False,
        debug=False,
        enable_asserts=False,
        num_devices=NCORES,
    )

    # per-superiter stream tensors, sup-major so each DMA is a fully
    # contiguous 2-dim [128, ~4-8KB] transfer (spreads descriptors over
    # all 16 SDMA engines).
    x2r = nc.dram_tensor("x2r", [NSUP // 2 * 128, 2 * TPS * FEAT], BF,
                         kind="ExternalInput").ap()
    x2t8 = nc.dram_tensor("x2t8", [NSUP // 2 * FEAT, 2 * X2TP], FP8,
                          kind="ExternalInput").ap()
    smalls = nc.dram_tensor("smalls", [128, SCOLS], BF,
                            kind="ExternalInput").ap()
    out_d = nc.dram_tensor("out", [BC, OUT], F32, kind="ExternalOutput").ap()

    with tile.TileContext(nc) as tc, ExitStack() as ctx:
        const = ctx.enter_context(tc.tile_pool(name="const", bufs=1))
        perst = ctx.enter_context(tc.tile_pool(name="perst", bufs=1))
        stream = ctx.enter_context(tc.tile_pool(name="stream", bufs=1))
        sm = ctx.enter_context(tc.tile_pool(name="sm", bufs=2))
        h1p = ctx.enter_context(tc.tile_pool(name="h1p", bufs=4))
        # PSUM: 8 bank-slots of 2KB.  en x2 + agg x2 + h1rps x2 + cen x2.
        psA = ctx.enter_context(tc.tile_pool(name="psA", bufs=2, space="PSUM"))
        psb = ctx.enter_context(tc.tile_pool(name="psb", bufs=2, space="PSUM"))
        psC = ctx.enter_context(tc.tile_pool(name="psC", bufs=2, space="PSUM"))

        sm_s = const.tile([128, SCOLS], BF, name="smalls")

        def sv(name, split=None):
            c = dict(SMALLS)[name]
            v = sm_s[:, SOFF[name]:SOFF[name] + c]
            if split is not None:
                v = v.rearrange("p (a b) -> p a b", a=split)
            return v

        x1tp = sv("x1tp")
        x0t = sv("x0t")
        x1r = sv("x1r", NTB)
        w0s4 = sv("w0s4")
        w0n4 = sv("w0n4")
        cn16 = sv("cn16", 4)
        cs16 = sv("cs16", 4)
        w0b = sv("w0b")
        w1b = sv("w1b", 4)
        wfcb = sv("wfcb", 4)
        e5p = sv("e5p")
        e5xp = sv("e5xp")
        L80p = sv("L80p")
        LBp = sv("LBp")
        ind16 = sv("ind16")
        indB16 = sv("indB16")
        i128b = sv("i128b")
        e10p = sv("e10p")
        e10xp = sv("e10xp")

        xc_sb = perst.tile([128, 4, NTB, 8, H], BF)  # stage-C agg (d-major)
        xb_bf = perst.tile([128, NTB, 8, H], BF)    # stage-B agg, f-major
        R_all = perst.tile([128, 16, 16, H], BF)    # stage-A e_s expander rhs
        RB = perst.tile([128, 16, H], BF)           # stage-B e_s expander rhs
        RC = perst.tile([128, 16, H], BF)           # stage-C e_s expander rhs

        x2t_t = [stream.tile([FEAT, 2, X2TP], FP8, tag=f"x2t{q}",
                             name=f"x2t{q}") for q in range(NSUP // 2)]
        x2r_t = [stream.tile([128, 2, TPS, FEAT], BF, tag=f"x2r{q}",
                             name=f"x2r{q}") for q in range(NSUP // 2)]

        mm = nc.tensor.matmul
        x2r_v = x2r.rearrange("(q p) c -> q p c", q=NSUP // 2)
        x2t_v = x2t8.rearrange("(q p) c -> q p c", q=NSUP // 2)

        # ---------- all DMAs upfront, consumption order ----------
        # 4 queues in parallel: SDMA engines round-robin between queues at
        # packet granularity, so concurrent queues hide per-descriptor HBM
        # latency.  smalls lead on the HWDGE queues (sync/scalar) so the
        # pre-loop PE work starts ASAP; x2r goes as 2MB pair-DMAs (16KB
        # descriptors).
        def dma_t(eng, s):
            eng.dma_start(x2t_t[s][:], x2t_v[s])

        def dma_r(eng, s):
            # one superiter = half of a pair tile; 8KB descriptors
            q, s2 = s // 2, s % 2
            eng.dma_start(
                x2r_t[q][:, s2, :, :].rearrange("p a b -> p (a b)"),
                x2r_v[q][:, s2 * TPS * FEAT:(s2 + 1) * TPS * FEAT])

        # Per-superiter DMAs sliced out of the pair tiles, interleaved
        # across the two HWDGE queues in consumption order (empirically the
        # fastest layout measured); non-critical smalls trail on gpsimd.
        def dma_t(eng, s):
            q, s2 = s // 2, s % 2
            eng.dma_start(x2t_t[q][:, s2, :],
                          x2t_v[q][:, s2 * X2TP:(s2 + 1) * X2TP])

        def dma_r(eng, s):
            q, s2 = s // 2, s % 2
            eng.dma_start(
                x2r_t[q][:, s2, :, :].rearrange("p a b -> p (a b)"),
                x2r_v[q][:, s2 * TPS * FEAT:(s2 + 1) * TPS * FEAT])

        def dma_tp(eng, q):
            eng.dma_start(x2t_t[q][:].rearrange("p a b -> p (a b)"),
                          x2t_v[q])

        # t0/t1 lead the scalar queue so L(0)/L(1) never wait behind the
        # smalls transfer on sync; r0 takes sync position 2.  Both queues
        # stay monotone in consumption order.
        nc.sync.dma_start(sm_s[:, :SC1], smalls[:, :SC1])
        dma_t(nc.sync, 0)
        dma_r(nc.scalar, 0)
        dma_t(nc.sync, 1)
        dma_r(nc.sync, 1)
        dma_t(nc.scalar, 2)
        dma_r(nc.scalar, 2)
        dma_t(nc.sync, 3)
        dma_r(nc.sync, 3)
        dma_t(nc.scalar, 4)
        dma_r(nc.scalar, 4)
        dma_t(nc.sync, 5)
        dma_r(nc.sync, 5)
        dma_t(nc.scalar, 6)
        dma_r(nc.scalar, 6)
        dma_t(nc.sync, 7)
        dma_r(nc.sync, 7)
        nc.gpsimd.dma_start(sm_s[:, SC1:SC2], smalls[:, SC1:SC2])
        nc.gpsimd.dma_start(sm_s[:, SC2:], smalls[:, SC2:])

        # ---------- pre-loop: stage-A e_s, stage-B e_s/e_n ----------
        # es80 chunk c: groups [80c, 80c+80).  R_all[g',c,t',h] =
        # e_s[80c+g', h] * [g' div 5 == t']
        es80_sb = sm.tile([128, 16, H], BF, tag="es80", bufs=1)
        for half in range(2):
            es_ps = psC.tile([128, 8, H], F32, tag="cen", name=f"es_ps{half}")
            for c_ in range(8):
                c = 8 * half + c_
                mm(es_ps[:, c_, :], x1tp[:, 80 * c:80 * c + 128], w0s4,
                   start=True, stop=True, skip_group_check=True)
            nc.vector.tensor_copy(es80_sb[:, 8 * half:8 * (half + 1), :],
                                  es_ps[:])
        nc.vector.tensor_mul(
            R_all[:],
            es80_sb[:].unsqueeze(2).broadcast_to((128, 16, 16, H)),
            ind16.unsqueeze(1).unsqueeze(3).broadcast_to((128, 16, 16, H)),
        )

        # esB = h0 @ w0s (one mm), RB = esB * indB16
        esB_ps = psC.tile([128, H], F32, tag="cen", name="esB_ps")
        mm(esB_ps[:], x0t, w0s4, start=True, stop=True, skip_group_check=True)
        esB_sb = sm.tile([128, H], BF, tag="esB", bufs=1)
        nc.vector.tensor_copy(esB_sb[:], esB_ps[:])
        nc.vector.tensor_mul(
            RB[:],
            esB_sb[:].unsqueeze(1).broadcast_to((128, 16, H)),
            indB16.unsqueeze(2).broadcast_to((128, 16, H)),
        )

        # stage-B logits
        enb_t = psA.tile([128, NTB, H], F32, tag="en", name="enb")
        mm(enb_t[:], LBp, RB[:], start=True, stop=False,
           skip_group_check=True)
        for i in range(NTB):
            mm(enb_t[:, i, :], x1tp[:, TB * i:TB * i + 128], w0n4,
               start=False, stop=True, skip_group_check=True)
        lrB = sm.tile([128, NTB, H], F32, tag="lr")
        nc.vector.tensor_scalar_mul(lrB[:], enb_t[:], LEAKY)
        nc.vector.tensor_max(lrB[:], lrB[:], enb_t[:])
        pB = sm.tile([128, NTB, H], BF, tag="p")
        nc.scalar.activation(pB[:], lrB[:], AF.Exp)
        mm(enb_t[:], e10xp, pB[:], start=True, stop=True,
           skip_group_check=True)
        rcB = sm.tile([128, NTB, H], F32, tag="rc")
        nc.vector.reciprocal_approx_fast(rcB[:], enb_t[:])
        alB = sm.tile([128, NTB, H], BF, tag="al")
        nc.gpsimd.tensor_mul(alB[:], pB[:], rcB[:])
        albdB = sm.tile([128, NTB, 8, H], BF, tag="albd")
        nc.gpsimd.tensor_mul(
            albdB[:],
            alB[:].unsqueeze(2).broadcast_to((128, NTB, 8, H)),
            e10p.unsqueeze(1).unsqueeze(3).broadcast_to((128, NTB, 8, H)),
        )

        # ---------- stage-A per-phase helpers ----------
        en_ts = [None] * NSUP     # logits psum tiles
        p_ts = [None] * NSUP      # exp'd logits
        albd_ts = [None] * NSUP
        xa_ts = [None] * NSUP     # x_aggA^T bf16
        h1r_ts = [None] * NSUP    # [chunk0, chunk1] row-major h1_new
        encp_ts = [None] * NSUP
        pc_ts = [None] * NSUP
        albdc_ts = [None] * NSUP

        def phase_L(s):
            en_t = psA.tile([128, TPS, H], F32, tag="en", name=f"en{s}")
            mm(en_t[:], L80p, R_all[:, 2 * s:2 * s + 2, :, :],
               start=True, stop=False, skip_group_check=True)
            for t in range(TPS):
                mm(en_t[:, t, :], x2t_t[s // 2][:, s % 2, 125 * t:125 * t + 128],
                   w0n4, start=False, stop=True, skip_group_check=True)
            en_ts[s] = en_t
            # softmax front half: leaky_relu (DVE) + exp (ACT)
            lr = sm.tile([128, TPS, H], F32, tag="lr", name=f"lr{s}")
            nc.vector.tensor_scalar_mul(lr[:], en_t[:], LEAKY)
            nc.vector.tensor_max(lr[:], lr[:], en_t[:])
            p = sm.tile([128, TPS, H], BF, tag="p", name=f"p{s}")
            nc.scalar.activation(p[:], lr[:], AF.Exp)
            p_ts[s] = p

        def phase_gsum(s):
            # group-sum reuses the logits PSUM region (logits dead after
            # exp); then the back half of softmax on DVE/GPS.
            en_t = en_ts[s]
            mm(en_t[:], e5xp, p_ts[s][:], start=True, stop=True,
               skip_group_check=True)
            rc = sm.tile([128, TPS, H], F32, tag="rc", name=f"rc{s}")
            nc.vector.reciprocal_approx_fast(rc[:], en_t[:])
            al = sm.tile([128, TPS, H], BF, tag="al", name=f"al{s}")
            albd = sm.tile([128, TPS, 5, H], BF, tag="albd", name=f"albd{s}")
            # t-halved so G's first 16 matmuls start off albd's first half
            # (subtile deps) ~1.2us earlier than a monolithic multiply
            HT = TPS // 2
            for hl in range(2):
                tsl = slice(HT * hl, HT * (hl + 1))
                nc.gpsimd.tensor_mul(al[:, tsl], p_ts[s][:, tsl], rc[:, tsl])
                nc.gpsimd.tensor_mul(
                    albd[:, tsl],
                    al[:, tsl].unsqueeze(2).broadcast_to((128, HT, 5, H)),
                    e5p.unsqueeze(1).unsqueeze(3).broadcast_to(
                        (128, HT, 5, H)),
                )
            albd_ts[s] = albd

        def phase_G(s):
            # aggregation: x_agg^T[f, (t, g, h)]
            albd = albd_ts[s]
            xa_bf = sm.tile([128, TPS, 5, H], BF, tag="xabf", name=f"xa{s}")
            for j in range(2):
                xa_ps = psb.tile([128, TPS // 2, 20], F32, tag="agg",
                                 name=f"xaps{s}_{j}")
                for t2 in range(16):
                    t = 16 * j + t2
                    mm(xa_ps[:, t2, :], x2r_t[s // 2][:TR, s % 2, t, :],
                       albd[:TR, t, :, :], start=True, stop=True,
                       skip_group_check=True)
                nc.scalar.copy(
                    xa_bf[:, 16 * j:16 * (j + 1), :, :].rearrange(
                        "p a b c -> p (a b c)"),
                    xa_ps[:].rearrange("p t x -> p (t x)"))
            xa_ts[s] = xa_bf

        def phase_Hfront(s):
            # h1_new row-major directly from xa_bf chunks (stationary),
            # with the stage-C e_n matmuls sharing each weight load.
            xa_bf = xa_ts[s]
            encp = psC.tile([128, 2, H], F32, tag="cen", name=f"encp{s}")
            mm(encp[:], LBp, RC[:, 2 * s:2 * s + 2, :], start=True,
               stop=False, skip_group_check=True)
            h1r_ts[s] = []
            for c in range(2):
                h1r_ps = psb.tile([128, H, 128], F32, tag="h1rps",
                                  name=f"h1rps{s}_{c}")
                for h1 in range(H):
                    stat = xa_bf[:, 16 * c:16 * (c + 1), :, h1]
                    mm(h1r_ps[:TB, h1, :], stat,
                       w0b[:, 128 * h1:128 * (h1 + 1)],
                       start=True, stop=True, skip_group_check=True)
                    mm(encp[:TB, c, :], stat, cn16[:, h1, :],
                       start=False, stop=(h1 == H - 1),
                       skip_group_check=True)
                h1r_sb = h1p.tile([128, H, 128], BF, tag="h1rsb",
                                  name=f"h1rsb{s}_{c}")
                nc.vector.tensor_copy(h1r_sb[:TB], h1r_ps[:TB])
                h1r_ts[s].append(h1r_sb)
            encp_ts[s] = encp
            lrc = sm.tile([128, 2, H], F32, tag="lrc", name=f"lrc{s}")
            nc.vector.tensor_scalar_mul(lrc[:], encp[:], LEAKY)
            nc.vector.tensor_max(lrc[:], lrc[:], encp[:])
            pc = sm.tile([128, 2, H], BF, tag="pc", name=f"pc{s}")
            nc.scalar.activation(pc[:], lrc[:], AF.Exp)
            pc_ts[s] = pc

        def phase_gsumC(s):
            encp = encp_ts[s]
            mm(encp[:], e10xp, pc_ts[s][:], start=True, stop=True,
               skip_group_check=True)
            rcc = sm.tile([128, 2, H], F32, tag="rcc", name=f"rcc{s}")
            nc.vector.reciprocal_approx_fast(rcc[:], encp[:])
            alc = sm.tile([128, 2, H], BF, tag="alc", name=f"alc{s}")
            nc.gpsimd.tensor_mul(alc[:], pc_ts[s][:], rcc[:])
            albdc = sm.tile([128, 2, 8, H], BF, tag="albdc",
                            name=f"albdc{s}")
            nc.gpsimd.tensor_mul(
                albdc[:],
                alc[:].unsqueeze(2).broadcast_to((128, 2, 8, H)),
                e10p.unsqueeze(1).unsqueeze(3).broadcast_to((128, 2, 8, H)),
            )
            albdc_ts[s] = albdc

        def phase_Cagg(s):
            albdc = albdc_ts[s]
            xc_ps = psb.tile([128, 2, 4, 8, H], F32, tag="agg",
                             name=f"xcps{s}")
            for c in range(2):
                for k in range(4):
                    mm(xc_ps[:, c, k, :, :], h1r_ts[s][c][:TB, k, :],
                       albdc[:TB, c, :, :], start=True, stop=True,
                       skip_group_check=True)
            nc.vector.tensor_copy(
                xc_sb[:, :, 2 * s:2 * s + 2, :, :].transpose([0, 2, 1, 3, 4]),
                xc_ps[:])

        # ---------- the skew-2 pipeline ----------
        for s in range(NSUP):
            phase_L(s)
            if s == 1:
                # stage-C e_s from xb_bf via fused Cs combos
                esC_ps = psC.tile([128, H], F32, tag="cen", name="esC_ps")
                for h1 in range(H):
                    mm(esC_ps[:], xb_bf[:, :, :, h1], cs16[:, h1, :],
                       start=(h1 == 0), stop=(h1 == H - 1),
                       skip_group_check=True)
                esC_sb = sm.tile([128, H], BF, tag="esC", bufs=1)
                nc.vector.tensor_copy(esC_sb[:], esC_ps[:])
                nc.vector.tensor_mul(
                    RC[:],
                    esC_sb[:].unsqueeze(1).broadcast_to((128, 16, H)),
                    indB16.unsqueeze(2).broadcast_to((128, 16, H)),
                )
            if s >= 1:
                phase_gsum(s - 1)
            if s >= 3:
                phase_Cagg(s - 3)
            if s >= 2:
                phase_Hfront(s - 2)
            if s == 0:
                # stage-B aggregation (albdB ready during L(0))
                xb_ps = psb.tile([128, NTB, 8, H], F32, tag="h1rps",
                                 name="xb_ps")
                for i in range(NTB):
                    mm(xb_ps[:, i, :, :], x1r[:, i, :], albdB[:, i, :, :],
                       start=True, stop=True, skip_group_check=True)
                nc.scalar.copy(xb_bf[:], xb_ps[:])
            if s >= 1:
                phase_G(s - 1)
            if s >= 2:
                phase_gsumC(s - 2)

        # ---------- drain + split epilogue ----------
        # Only batches 112-127 (xc tiles 14,15) depend on the last
        # superiter: run the epilogue for batches 0-111 concurrently with
        # the s=7 drain phases, and a small tail for the rest.
        # epilogue PSUM lives in tags that are DEAD during the drain:
        # en (psA) frees after gsum(7); or_ps slots into the agg rotation
        # after xc(6).
        hf_bf = sm.tile([128, H, BC], BF, tag="hfbf", bufs=1)
        ot_bf = sm.tile([128, 2, BC], BF, tag="otbf", bufs=1)
        or_sb = sm.tile([BC, 2, 128], F32, tag="orsb", bufs=1)

        def epilogue(b0, b1, t0, t1):
            # hf/FC columns [b0:b1); output transpose + store rows [t0:t1)
            # (transpose out base partition must be 0/32/64, so the
            # early/late split differs between the two halves).
            for h in range(H):
                for k in range(4):
                    mm(hf_ps[:, h, b0:b1], w1b[:, k, 128 * h:128 * (h + 1)],
                       xc_sb[:, k, :, :, h].rearrange(
                           "p a b -> p (a b)")[:, b0:b1],
                       start=(k == 0), stop=(k == 3), skip_group_check=True)
            nc.scalar.copy(hf_bf[:, :, b0:b1], hf_ps[:, :, b0:b1])
            for m in range(2):
                for k in range(4):
                    mm(of_ps[:, m, b0:b1], wfcb[:, k, 128 * m:128 * (m + 1)],
                       hf_bf[:, k, b0:b1], start=(k == 0), stop=(k == 3),
                       skip_group_check=True)
            nc.vector.tensor_copy(ot_bf[:, :, b0:b1], of_ps[:, :, b0:b1])
            for m in range(2):
                mm(or_ps[t0:t1, m, :], ot_bf[:, m, t0:t1], i128b,
                   start=True, stop=True, skip_group_check=True)
            nc.vector.tensor_copy(or_sb[t0:t1], or_ps[t0:t1])
            nc.sync.dma_start(
                out_d.rearrange("b (m o) -> b m o", m=2)[t0:t1],
                or_sb[t0:t1])

        phase_gsum(NSUP - 1)
        phase_Cagg(NSUP - 3)
        phase_Hfront(NSUP - 2)
        phase_G(NSUP - 1)
        phase_gsumC(NSUP - 2)
        phase_Cagg(NSUP - 2)
        hf_ps = psA.tile([128, H, BC], F32, tag="en", name="hf_ps")
        of_ps = psA.tile([128, 2, BC], F32, tag="en", name="of_ps")
        or_ps = psb.tile([BC, 2, 128], F32, tag="agg", name="or_ps")
        epilogue(0, 112, 0, 64)
        phase_Hfront(NSUP - 1)
        phase_gsumC(NSUP - 1)
        phase_Cagg(NSUP - 1)
        epilogue(112, 128, 64, 128)

        if debug_out:
            dbg_xb = nc.dram_tensor("dbg_xb", [128, NTB * 8 * H], BF,
                                    kind="ExternalOutput").ap()
            dbg_xc = nc.dram_tensor("dbg_xc", [128, 4 * NTB * 8 * H], BF,
                                    kind="ExternalOutput").ap()
            dbg_h1r = nc.dram_tensor("dbg_h1r", [128, H * 128], BF,
                                     kind="ExternalOutput").ap()
            nc.sync.dma_start(
                dbg_xb.rearrange("p (a b c) -> p a b c", a=NTB, b=8), xb_bf[:])
            nc.sync.dma_start(
                dbg_xc.rearrange("p (a b c d) -> p a b c d", a=4, b=NTB, c=8),
                xc_sb[:])
            nc.sync.dma_start(
                dbg_h1r.rearrange("p (a b) -> p a b", a=H), h1r_ts[0][0][:])

    nc.compile()
    return nc


def _host_prep(h0, h1, h2, W0, a0_s, a0_n, W1, a1_s, a1_n, W_fc):
    bf16 = ml_dtypes.bfloat16
    fp8 = ml_dtypes.float8_e4m3
    f32 = np.float32

    def combo(W, a):  # [F, H*D], [H, D] -> [F, H]
        F_ = W.shape[0]
        return np.einsum("fhd,hd->fh", W.reshape(F_, H, 128), a).astype(f32)

    w0s = combo(W0, a0_s)
    w0n = combo(W0, a0_n)
    w1s = combo(W1, a1_s)                                         # [512, H]
    w1n = combo(W1, a1_n)
    # fused layer-1 combos: Cn[f, h1, h] = sum_d W0[f,h1*128+d] w1n[h1*128+d,h]
    W0r = np.asarray(W0, f32).reshape(FEAT, H, 128)
    cn = np.einsum("fad,adh->fah", W0r, w1n.reshape(H, 128, H))   # [128,4,H]
    cs = np.einsum("fad,adh->fah", W0r, w1s.reshape(H, 128, H))

    ar = np.arange
    sm_shared = {}

    def put(name, arr):
        a = np.zeros((128, dict(SMALLS)[name]), dtype=bf16)
        a[:arr.shape[0], :arr.shape[1]] = arr.astype(bf16)
        sm_shared[name] = a

    put("w0s4", w0s)
    put("w0n4", w0n)
    put("cn16", cn.reshape(128, 4 * H))
    put("cs16", cs.reshape(128, 4 * H))
    put("w0b", W0.astype(f32))
    put("w1b", W1.reshape(4, 128, HID).transpose(1, 0, 2).reshape(128, -1))
    put("wfcb", W_fc.reshape(4, 128, OUT).transpose(1, 0, 2).reshape(128, -1))
    e5p = (ar(128)[:, None] // 25 == ar(5)[None, :]) & (ar(128)[:, None] < 125)
    put("e5p", e5p.astype(f32))
    e5x = np.zeros((128, 128), dtype=f32)
    blk = (ar(125)[:, None] // 25 == ar(125)[None, :] // 25)
    e5x[:125, :125] = blk
    e5x[125:, :] = 0.0
    for m in range(125, 128):
        e5x[m, m] = 1.0
    put("e5xp", e5x)
    L80 = np.zeros((128, 128), dtype=f32)
    L80[:80, :125] = (ar(80)[:, None] % 5 == ar(125)[None, :] // 25)
    put("L80p", L80)
    LB = np.zeros((128, 128), dtype=f32)
    LB[:, :80] = (ar(128)[:, None] % 8 == ar(80)[None, :] // 10)
    put("LBp", LB)
    ind16 = np.zeros((128, 16), dtype=f32)
    ind16[:80] = (ar(80)[:, None] // 5 == ar(16)[None, :])
    put("ind16", ind16)
    put("indB16", (ar(128)[:, None] // 8 == ar(16)[None, :]).astype(f32))
    put("i128b", np.eye(128, dtype=f32))
    e10 = np.zeros((128, 8), dtype=f32)
    e10[:80] = (ar(80)[:, None] // 10 == ar(8)[None, :])
    put("e10p", e10)
    e10x = np.zeros((128, 128), dtype=f32)
    e10x[:80, :80] = (ar(80)[:, None] // 10 == ar(80)[None, :] // 10)
    for m in range(80, 128):
        e10x[m, m] = 1.0
    put("e10xp", e10x)

    in_maps = []
    for c in range(NCORES):
        sl = slice(c * BC, (c + 1) * BC)
        h2c = np.asarray(h2[sl], dtype=f32).reshape(R2, FEAT)
        h1c = np.asarray(h1[sl], dtype=f32).reshape(G1, FEAT)
        h0c = np.asarray(h0[sl], dtype=f32)
        m = dict(sm_shared)
        x1tp = np.zeros((128, G1P), dtype=bf16)
        x1tp[:, :G1] = h1c.T.astype(bf16)
        m["x1tp"] = x1tp
        m["x0t"] = np.zeros((128, BC), dtype=bf16)
        m["x0t"][:] = h0c.T.astype(bf16)
        x1r = np.zeros((128, NTB, FEAT), dtype=bf16)
        x1r[:TB] = h1c.astype(bf16).reshape(NTB, TB, FEAT).transpose(1, 0, 2)
        m["x1r"] = x1r.reshape(128, NTB * FEAT)
        # pack the smalls in layout order
        packed = np.concatenate([m.pop(n) for n, _ in SMALLS], axis=1)
        mm_ = {"smalls": np.ascontiguousarray(packed)}
        # x2r: [pair q, 128 partitions (125 used), 2 sup, TPS, FEAT]
        x2rt = h2c.astype(bf16).reshape(NSUP // 2, 2, TPS, TR, FEAT) \
            .transpose(0, 3, 1, 2, 4)                # [q, TR, 2, TPS, F]
        x2rp = np.zeros((NSUP // 2, 128, 2, TPS, FEAT), dtype=bf16)
        x2rp[:, :TR] = x2rt
        mm_["x2r"] = np.ascontiguousarray(
            x2rp.reshape(NSUP // 2 * 128, 2 * TPS * FEAT))
        # x2t: [pair q, FEAT, 2 sup, X2TP] feature-major fp8 (8KB rows)
        x2t = np.zeros((NSUP // 2, FEAT, 2, X2TP), dtype=fp8)
        x2t[:, :, :, :SUPR] = h2c.T.astype(fp8).reshape(
            FEAT, NSUP // 2, 2, SUPR).transpose(1, 0, 2, 3)
        mm_["x2t8"] = np.ascontiguousarray(
            x2t.reshape(NSUP // 2 * FEAT, 2 * X2TP))
        in_maps.append(mm_)
    return in_maps


_PROGRAM = None


def kernel(**inputs):
    global _PROGRAM
    if _PROGRAM is None:
        _PROGRAM = build_program()
    in_maps = _host_prep(**{k: np.asarray(v) for k, v in inputs.items()})
    res = bass_utils.run_bass_kernel_spmd(
        _PROGRAM, in_maps, core_ids=list(range(NCORES)))
    return np.concatenate([r["out"] for r in res.results], axis=0)


if __name__ == "__main__":
    build_program()
    print("program built + compiled OK")
